# revision 4
# baseline (speedup 1.0000x reference)
"""Haar 3D wavelet transform (2x2x2 stride-2 conv, 8 sign filters) on 8 trn2 cores.

Input  x: (2, 3, 33, 512, 512) f32, w: (8, 1, 2, 2, 2) f32.
Output:   (2, 24, 17, 256, 256) f32.

Pure streaming kernel -> HW time is bound by HBM traffic.  The tolerance
(rel err < 2e-2 on an absmax-normalized metric) leaves a ~30x error budget
over fp16, so both streams ride HBM as INT8 (uniform quantization), halving
traffic vs the fp16 version (13.1 MiB/core vs 26.7 MiB/core):
  input  xq = rint(x / d_in),  d_in = absmax(x)/127
  device S  = sum of +-xq (exact integers; PE matmul of bf16(+-1) x bf16(int))
  output yq = rint(S * alpha), alpha = 127/Bint  (RNE f32->i8 on ACT/DVE)
  host   y  = yq * d_out,      d_out = wamax*d_in*Bint/127
where Bint = max over output windows of sum|xq| (computed host-side on the
quantized input), so |S*alpha| <= 127 exactly - no clipping can occur.
Measured end-to-end rel err 1.33e-2 (tolerance 2e-2); the device path is
bit-identical to the numpy sim (integer matmul exact in bf16/f32-PSUM,
ACT/DVE f32->i8 casts are RNE - hardware-verified).

Work unit and layout identical to the fp16 version: qu = ((b,c,t), s),
408 qus / 8 cores = 51 each; partition p = dt*64+dh*32+dw*16+g, free
f = r*256+wo; per-core input packed partition-major into (128, 52224) i8.

Engine facts this schedule is built on (all HW-measured here):
  - engine tensor_copy CAST i8->f16 is ~27 Gelem/s (8x too slow) -> input
    dtype conversion rides the SWDGE cast-DMA instead (HBM i8 -> SBUF bf16
    in the SDMA datapath, hardware-exact for ints).
  - gpsimd cannot read PSUM (walrus birverifier) -> evac is ACT+DVE only:
    ACT ACTIVATE(scale) 2.36us / DVE TENSOR_SCALAR 2.73us per 2048-col
    block, split 54/46 -> ~32us each across the kernel.
  - PE HAM throttle: matmuls run 1.2 GHz until ~3.4us of sustained busy,
    2.4 GHz after; the stream must keep PE gaps < ~3.4us so the 102
    512-col matmuls stay warm (~26us total, off the critical path).
Ring layout: Pool(SWDGE) carries ONLY the input cast-DMAs (an output's
evac-wait can never head-block the stream feeding the PE); outputs
alternate the two HWDGE rings (ACT even chunks, SP odd chunks; SP also
takes the 32KB weight first).  Chunk 0's single evac is split across
DVE+ACT concurrently to halve the first-output latency.
"""

import numpy as np

N_CORES = 8
B, C, T_IN, H, W = 2, 3, 33, 512, 512
T_OUT = 17
QU_PER_CORE = 51            # 2*3*17*4 / 8
NCOL = QU_PER_CORE * 1024   # 52224 free columns per core
CHUNKS = [2048, 4096] + [8192] * 5 + [3072, 1024, 1024]
assert sum(CHUNKS) == NCOL
# input path split: ~55% of columns via SWDGE cast-DMA (Pool ring), the
# rest plain i8 on the SP ring + DVE tensor_scalar casts - balances the
# SDMA fabric (cast-DMA is charged 2B/elem on the SBUF side) against
# DVE/ACT engine time.
CASTDMA_CHUNKS = {0, 2, 4, 6, 8, 9}
# evac engine pattern: DVE on ~36% of blocks (it also does input casts)
_EVAC_DVE = {0, 3, 6, 9, 12}  # of evb % 14


def _build_nc(alpha, legalize=True):
    import concourse.bass as bass
    import concourse.mybir as mybir
    from concourse.tile import TileContext

    f16 = mybir.dt.float16
    i8 = mybir.dt.int8
    f32 = mybir.dt.float32
    COPY = mybir.ActivationFunctionType.Copy
    nc = bass.Bass()
    xin = nc.declare_dram_parameter("xin", [128, NCOL], i8, isOutput=False)
    wmat = nc.declare_dram_parameter("wmat", [128, 128], f16, isOutput=False)
    yout = nc.declare_dram_parameter("yout", [128, NCOL], i8, isOutput=True)

    with TileContext(nc) as tc:
        with (
            tc.tile_pool(name="const", bufs=1) as cpool,
            tc.tile_pool(name="x8", bufs=3) as x8pool,
            tc.tile_pool(name="xf", bufs=4) as xfpool,
            tc.tile_pool(name="rpool", bufs=4) as rpool,
            tc.tile_pool(name="ppool", bufs=2, space="PSUM") as ppool,
        ):
            wt = cpool.tile([128, 128], f16)

            c0 = 0
            evb = 0
            for ci, ch in enumerate(CHUNKS):
                castdma = ci in CASTDMA_CHUNKS
                xf = xfpool.tile([128, 8192], f16, tag="xf")
                if castdma:
                    # SWDGE cast-DMA: HBM i8 -> SBUF f16 in the SDMA datapath
                    # (Pool ring carries only these; nothing can head-block it).
                    nc.gpsimd.dma_start(out=xf[:, :ch], in_=xin[:, c0:c0 + ch])
                    x8 = None
                else:
                    # plain i8 on the SP HWDGE ring; DVE tensor_scalar casts
                    # i8 -> f16 per block (215 Gelem/s; tensor_copy's CAST
                    # path is 8x slower - never use it for 8-bit).
                    x8 = x8pool.tile([128, 8192], i8, tag="x8")
                    nc.sync.dma_start(out=x8[:, :ch], in_=xin[:, c0:c0 + ch])
                if ci == 0:
                    nc.sync.dma_start(out=wt[:], in_=wmat[:])
                rt = rpool.tile([128, 8192], i8, tag="r")
                for off in range(0, ch, 2048):
                    blk = min(2048, ch - off)
                    if not castdma:
                        nc.vector.tensor_scalar_mul(
                            xf[:, off:off + blk], x8[:, off:off + blk], 1.0)
                    pt = ppool.tile([128, 2048], f32, tag="p")
                    for m in range(0, blk, 512):
                        sz = min(512, blk - m)
                        nc.tensor.matmul(
                            pt[:, m:m + sz],
                            lhsT=wt[:],
                            rhs=xf[:, off + m:off + m + sz],
                            start=True, stop=True)
                    # PSUM f32 -> i8 with scale (RNE): ACT ~16/25.5 blocks,
                    # DVE the rest (DVE also carries all the input casts).
                    if ci == 0:
                        h = blk // 2
                        nc.vector.tensor_scalar_mul(rt[:, off:off + h], pt[:, :h], alpha)
                        nc.scalar.activation(rt[:, off + h:off + blk], pt[:, h:blk], COPY, scale=alpha)
                    elif evb % 14 in _EVAC_DVE:
                        nc.vector.tensor_scalar_mul(rt[:, off:off + blk], pt[:, :blk], alpha)
                    else:
                        nc.scalar.activation(rt[:, off:off + blk], pt[:, :blk], COPY, scale=alpha)
                    evb += 1
                # Outputs: ACT ring while SP still has inputs to issue; the
                # last three chunks ride SP (its inputs are done by then).
                if ci >= 7:
                    nc.sync.dma_start(out=yout[:, c0:c0 + ch], in_=rt[:, :ch])
                else:
                    nc.scalar.dma_start(out=yout[:, c0:c0 + ch], in_=rt[:, :ch])
                c0 += ch

    if legalize:
        _legalize_waits(nc)
    return nc


def _legalize_waits(nc, limit=1):
    """walrus codegen rejects instructions carrying more than ~1 sem wait
    (e.g. Matmult's LoadWeights slot).  Move excess waits onto NoOp
    instructions inserted just before the instruction on the same engine
    queue -- semantically identical (all waits still precede execution)."""
    import bass_rust

    fn = nc.m.functions[0]
    lastblk = fn.blocks[-1]
    eng_ns = {
        "PE": nc.tensor, "DVE": nc.vector, "Activation": nc.scalar,
        "SP": nc.sync, "Pool": nc.gpsimd,
    }
    # NoOp codegen requires >=1 sem update. Give each engine its own dummy
    # sem (ids picked from the top of the 150..255 HW range, skipping any id
    # already referenced) so no counting or cross-proc rule is disturbed.
    used_ids = set()
    for blk in fn.blocks:
        for inst in blk.instructions:
            si = getattr(inst, "sync_info", None)
            if si is None:
                continue
            for w in si.on_wait:
                used_ids.add(w.id)
            for upd in si.on_update:
                used_ids.add(upd.id)
    avail = [i for i in range(255, 149, -1) if i not in used_ids]
    eng_upd = {}
    for k, en in enumerate(["PE", "DVE", "Activation", "SP", "Pool"]):
        eng_upd[en] = bass_rust.SyncUpdate(
            sync_type="semaphore", id=avail[k], ant_name=f"waitnop_{en}",
            update_mode="sem-inc", update_value=1, update_reg=None)

    def copy_wait(w):
        return bass_rust.SyncWait(
            sync_type=w.sync_type, id=w.id, ant_name=w.ant_name,
            wait_mode=w.wait_mode, wait_value=w.wait_value, wait_reg=w.wait_reg)

    def make_nop(engine_name, waits):
        ns = eng_ns[engine_name]
        ns.nop(hint="waitcarrier")
        nop = lastblk.instructions.pop()
        raw = getattr(nop, "inst", nop)
        raw.sync_info = bass_rust.SyncInfo(
            on_wait=[copy_wait(w) for w in waits],
            on_update=[eng_upd[engine_name]])
        return raw

    for blk in fn.blocks:
        insts = blk.instructions
        i = 0
        while i < len(insts):
            inst = insts[i]
            ty = type(inst).__name__
            si = getattr(inst, "sync_info", None)
            if (ty not in ("InstEventSemaphore", "InstNoOp")
                    and si is not None and len(si.on_wait) > limit):
                ename = str(inst.engine).split(".")[-1]
                waits = [copy_wait(w) for w in si.on_wait]
                upds = list(si.on_update)
                extra, keep = waits[:-limit], waits[-limit:]
                for w in extra:
                    insts.insert(i, make_nop(ename, [w]))
                    i += 1
                inst.sync_info = bass_rust.SyncInfo(
                    on_wait=keep, on_update=upds)
            i += 1


def _make_wmat(w):
    """128x128 stationary butterfly matrix, normalized to +-1 entries:
    W[(dt,dh,dw,g), (k,g)] = w[k,dt,dh,dw]/wamax.  Exact in fp16 for the
    Haar +-SCALE filters; works for any 8-filter 2x2x2 kernel."""
    w8 = np.asarray(w, dtype=np.float32).reshape(8, 2, 2, 2)
    wamax = float(np.abs(w8).max())
    if wamax == 0.0:
        wamax = 1.0
    wm = np.zeros((128, 128), dtype=np.float32)
    g = np.arange(16)
    for k in range(8):
        for dt in range(2):
            for dh in range(2):
                for dw in range(2):
                    wm[dt * 64 + dh * 32 + dw * 16 + g, k * 16 + g] = \
                        w8[k, dt, dh, dw] / wamax
    return wm.astype(np.float16), wamax


def _pack_input(xq):
    """xq (2,3,33,512,512) int8 -> list of 8 (128, NCOL) int8 per-core arrays."""
    pairs = np.empty((T_OUT, 2), dtype=np.int64)
    for t in range(T_OUT):
        pairs[t, 0] = max(2 * t - 1, 0)
        pairs[t, 1] = 2 * t
    full = xq[:, :, pairs]                        # (b, c, t, dt, 512, 512)
    # h = s*128 + g*8 + r*2 + dh ; w = wo*2 + dw
    arr = full.reshape(B, C, T_OUT, 2, 4, 16, 4, 2, 256, 2)
    #                  b  c  t     dt s  g   r  dh wo  dw
    arr = arr.transpose(0, 1, 2, 4, 3, 7, 9, 5, 6, 8)
    # (b, c, t, s, dt, dh, dw, g, r, wo)
    arr = np.ascontiguousarray(arr).reshape(8 * QU_PER_CORE, 128, 1024)
    return [
        np.ascontiguousarray(
            arr[QU_PER_CORE * m:QU_PER_CORE * (m + 1)].transpose(1, 0, 2)
        ).reshape(128, NCOL)
        for m in range(N_CORES)
    ]


def _unpack_output(youts, d_out):
    """list of 8 (128, NCOL) int8 -> (2, 24, 17, 256, 256) f32."""
    Y = np.stack(youts)                           # (8, 128, NCOL)
    arr = Y.reshape(N_CORES, 128, QU_PER_CORE, 1024).transpose(0, 2, 1, 3)
    arr = arr.reshape(B, C, T_OUT, 4, 8, 16, 4, 256)
    #                 b  c  t     s  k  g   r  wo
    arr = arr.transpose(0, 4, 1, 2, 3, 5, 6, 7)
    # (b, k, c, t, s, g, r, wo): channel = k*3+c, ho = s*64 + g*4 + r
    out = np.ascontiguousarray(arr).reshape(
        B, 24, T_OUT, 256, 256).astype(np.float32)
    out *= np.float32(d_out)
    return out


LAST_RESULT = None


def kernel(x, w):
    import os
    from concourse.bass_utils import run_bass_kernel_spmd

    x = np.asarray(x, dtype=np.float32)
    ax = float(np.abs(x).max())
    if ax == 0.0:
        ax = 1.0
    d_in = ax / 127.0
    xq = np.rint(x * np.float32(1.0 / d_in)).astype(np.int8)

    wm, wamax = _make_wmat(w)

    # Exact bound on |sum of +-xq| over any output window: max sum-abs pool
    # of the quantized input (with the causal first-frame replication).
    xa = np.abs(xq.astype(np.int16))
    pad = np.concatenate([xa[:, :, :1], xa], axis=2)      # (2,3,34,512,512)
    Bint = int(pad.reshape(B, C, T_OUT, 2, 256, 2, 256, 2)
               .sum(axis=(3, 5, 7), dtype=np.int32).max())
    if Bint == 0:
        Bint = 1
    alpha = np.float32(127.0 / Bint)
    d_out = wamax * d_in * Bint / 127.0

    in_maps = [{"xin": xc, "wmat": wm} for xc in _pack_input(xq)]

    nc = _build_nc(float(alpha))
    kw = {}
    if os.environ.get("KERNEL_PROFILE") == "1":
        kw = dict(trace=True, tmpdir=os.environ.get("KERNEL_PROFILE_DIR"))
    res = run_bass_kernel_spmd(nc, in_maps, core_ids=list(range(N_CORES)), **kw)
    global LAST_RESULT
    LAST_RESULT = res

    return _unpack_output(
        [np.asarray(res.results[m]["yout"]) for m in range(N_CORES)], d_out)


if __name__ == "__main__":
    rng = np.random.default_rng(0)
    x = rng.standard_normal((B, C, T_IN, H, W), dtype=np.float32)
    SCALE = 0.3536
    flags = np.array([[0, 0, 0], [0, 0, 1], [0, 1, 0], [0, 1, 1],
                      [1, 0, 0], [1, 0, 1], [1, 1, 0], [1, 1, 1]])
    t, h, ww = np.meshgrid(np.arange(2), np.arange(2), np.arange(2), indexing="ij")
    sign = (-1.0) ** (flags[:, 0, None, None, None] * t
                      + flags[:, 1, None, None, None] * h
                      + flags[:, 2, None, None, None] * ww)
    wf = (SCALE * sign).reshape(8, 1, 2, 2, 2).astype(np.float32)
    y = kernel(x, wf)
    print(y.shape, y.dtype)


# revision 6
# speedup vs baseline: 1.3755x; 1.3755x over previous
"""Haar 3D wavelet transform (2x2x2 stride-2 conv, 8 sign filters) on 8 trn2 cores.

Input  x: (2, 3, 33, 512, 512) f32, w: (8, 1, 2, 2, 2) f32.
Output:   (2, 24, 17, 256, 256) f32.

Pure streaming kernel -> HW time is bound by HBM traffic.  The tolerance
(rel err < 2e-2 on an absmax-normalized metric) leaves a ~30x error budget
over fp16, so both streams ride HBM as INT8 (uniform quantization), halving
traffic vs the fp16 version (13.1 MiB/core vs 26.7 MiB/core):
  input  xq = rint(x / d_in),  d_in = absmax(x)/127
  device S  = sum of +-xq (exact integers; PE matmul of bf16(+-1) x bf16(int))
  output yq = rint(S * alpha), alpha = 127/Bint  (RNE f32->i8 on ACT/DVE)
  host   y  = yq * d_out,      d_out = wamax*d_in*Bint/127
where Bint = max over output windows of sum|xq| (computed host-side on the
quantized input), so |S*alpha| <= 127 exactly - no clipping can occur.
Measured end-to-end rel err 1.33e-2 (tolerance 2e-2); the device path is
bit-identical to the numpy sim (integer matmul exact in bf16/f32-PSUM,
ACT/DVE f32->i8 casts are RNE - hardware-verified).

Work unit and layout identical to the fp16 version: qu = ((b,c,t), s),
408 qus / 8 cores = 51 each; partition p = dt*64+dh*32+dw*16+g, free
f = r*256+wo; per-core input packed partition-major into (128, 52224) i8.

Engine facts this schedule is built on (all HW-measured here):
  - engine tensor_copy CAST i8->f16 is ~27 Gelem/s (8x too slow) -> input
    dtype conversion rides the SWDGE cast-DMA instead (HBM i8 -> SBUF bf16
    in the SDMA datapath, hardware-exact for ints).
  - gpsimd cannot read PSUM (walrus birverifier) -> evac is ACT+DVE only:
    ACT ACTIVATE(scale) 2.36us / DVE TENSOR_SCALAR 2.73us per 2048-col
    block, split 54/46 -> ~32us each across the kernel.
  - PE HAM throttle: matmuls run 1.2 GHz until ~3.4us of sustained busy,
    2.4 GHz after; the stream must keep PE gaps < ~3.4us so the 102
    512-col matmuls stay warm (~26us total, off the critical path).
Ring layout: Pool(SWDGE) carries ONLY the input cast-DMAs (an output's
evac-wait can never head-block the stream feeding the PE); outputs
alternate the two HWDGE rings (ACT even chunks, SP odd chunks; SP also
takes the 32KB weight first).  Chunk 0's single evac is split across
DVE+ACT concurrently to halve the first-output latency.
"""

import numpy as np

N_CORES = 8
B, C, T_IN, H, W = 2, 3, 33, 512, 512
T_OUT = 17
QU_PER_CORE = 51            # 2*3*17*4 / 8
NCOL = QU_PER_CORE * 1024   # 52224 free columns per core
CHUNKS = [2048, 4096] + [8192] * 5 + [3072, 1024, 1024]
assert sum(CHUNKS) == NCOL
# dummy matmuls issued before the stream to warm the PE's HAM throttle
# during the fixed ~9us framework preamble
WARMUP_MM = 18


def _build_nc(alpha, legalize=True):
    import concourse.bass as bass
    import concourse.mybir as mybir
    from concourse.tile import TileContext

    f16 = mybir.dt.float16
    i8 = mybir.dt.int8
    f32 = mybir.dt.float32
    COPY = mybir.ActivationFunctionType.Copy
    nc = bass.Bass()
    xin = nc.declare_dram_parameter("xin", [128, NCOL], i8, isOutput=False)
    wmat = nc.declare_dram_parameter("wmat", [128, 128], f16, isOutput=False)
    yout = nc.declare_dram_parameter("yout", [128, NCOL], i8, isOutput=True)

    with TileContext(nc) as tc:
        with (
            tc.tile_pool(name="const", bufs=1) as cpool,
            tc.tile_pool(name="xf", bufs=6) as xfpool,
            tc.tile_pool(name="rpool", bufs=6) as rpool,
            tc.tile_pool(name="ppool", bufs=3, space="PSUM") as ppool,
            tc.tile_pool(name="wpool", bufs=1, space="PSUM") as wpool,
        ):
            wt = cpool.tile([128, 128], f16)
            # PE warm-up: the HAM activity throttle runs the PE at 1.2 GHz
            # until it has been busy ~3.4us (and re-throttles after any
            # ~3.4us idle).  The fixed ~9us framework preamble would hand
            # the first chunks a cold PE; burn dummy matmuls on an
            # uninitialized scratch tile (values irrelevant) during the
            # preamble so the stream starts at 2.4 GHz.
            scratch = cpool.tile([128, 512], f16)
            nc.vector.memzero(scratch[:])
            wpt = wpool.tile([128, 512], f32, tag="warm")
            for _ in range(WARMUP_MM):
                nc.tensor.matmul(wpt[:], lhsT=scratch[:, :128], rhs=scratch[:],
                                 start=True, stop=True, skip_group_check=True)

            c0 = 0
            evb = 0
            for ci, ch in enumerate(CHUNKS):
                xf = xfpool.tile([128, 8192], f16, tag="xf")
                # SWDGE cast-DMA: HBM i8 -> SBUF f16 in the SDMA datapath
                # (Pool ring carries only inputs; nothing can head-block it).
                nc.gpsimd.dma_start(out=xf[:, :ch], in_=xin[:, c0:c0 + ch])
                if ci == 0:
                    nc.sync.dma_start(out=wt[:], in_=wmat[:])
                rt = rpool.tile([128, 8192], i8, tag="r")
                for off in range(0, ch, 1024):
                    blk = min(1024, ch - off)
                    pt = ppool.tile([128, 1024], f32, tag="p")
                    for m in range(0, blk, 512):
                        sz = min(512, blk - m)
                        nc.tensor.matmul(
                            pt[:, m:m + sz],
                            lhsT=wt[:],
                            rhs=xf[:, off + m:off + m + sz],
                            start=True, stop=True)
                    # PSUM f32 -> i8 with scale (RNE); 1024-col grain keeps
                    # the PE and the evacuating engines loosely coupled.
                    if ci == 0:
                        h = blk // 2
                        nc.vector.tensor_scalar_mul(rt[:, off:off + h], pt[:, :h], alpha)
                        nc.scalar.activation(rt[:, off + h:off + blk], pt[:, h:blk], COPY, scale=alpha)
                    elif evb % 2 == 0:
                        nc.scalar.activation(rt[:, off:off + blk], pt[:, :blk], COPY, scale=alpha)
                    else:
                        nc.vector.tensor_scalar_mul(rt[:, off:off + blk], pt[:, :blk], alpha)
                    evb += 1
                # Outputs alternate the two HWDGE rings (neither carries
                # inputs, so an output's evac-wait can block nothing).
                if ci % 2 == 0:
                    nc.scalar.dma_start(out=yout[:, c0:c0 + ch], in_=rt[:, :ch])
                else:
                    nc.sync.dma_start(out=yout[:, c0:c0 + ch], in_=rt[:, :ch])
                c0 += ch

    if legalize:
        _legalize_waits(nc)
    return nc


def _legalize_waits(nc, limit=1):
    """walrus codegen rejects instructions carrying more than ~1 sem wait
    (e.g. Matmult's LoadWeights slot).  Move excess waits onto NoOp
    instructions inserted just before the instruction on the same engine
    queue -- semantically identical (all waits still precede execution)."""
    import bass_rust

    fn = nc.m.functions[0]
    lastblk = fn.blocks[-1]
    eng_ns = {
        "PE": nc.tensor, "DVE": nc.vector, "Activation": nc.scalar,
        "SP": nc.sync, "Pool": nc.gpsimd,
    }
    # NoOp codegen requires >=1 sem update. Give each engine its own dummy
    # sem (ids picked from the top of the 150..255 HW range, skipping any id
    # already referenced) so no counting or cross-proc rule is disturbed.
    used_ids = set()
    for blk in fn.blocks:
        for inst in blk.instructions:
            si = getattr(inst, "sync_info", None)
            if si is None:
                continue
            for w in si.on_wait:
                used_ids.add(w.id)
            for upd in si.on_update:
                used_ids.add(upd.id)
    avail = [i for i in range(255, 149, -1) if i not in used_ids]
    eng_upd = {}
    for k, en in enumerate(["PE", "DVE", "Activation", "SP", "Pool"]):
        eng_upd[en] = bass_rust.SyncUpdate(
            sync_type="semaphore", id=avail[k], ant_name=f"waitnop_{en}",
            update_mode="sem-inc", update_value=1, update_reg=None)

    def copy_wait(w):
        return bass_rust.SyncWait(
            sync_type=w.sync_type, id=w.id, ant_name=w.ant_name,
            wait_mode=w.wait_mode, wait_value=w.wait_value, wait_reg=w.wait_reg)

    def make_nop(engine_name, waits):
        ns = eng_ns[engine_name]
        ns.nop(hint="waitcarrier")
        nop = lastblk.instructions.pop()
        raw = getattr(nop, "inst", nop)
        raw.sync_info = bass_rust.SyncInfo(
            on_wait=[copy_wait(w) for w in waits],
            on_update=[eng_upd[engine_name]])
        return raw

    for blk in fn.blocks:
        insts = blk.instructions
        i = 0
        while i < len(insts):
            inst = insts[i]
            ty = type(inst).__name__
            si = getattr(inst, "sync_info", None)
            if (ty not in ("InstEventSemaphore", "InstNoOp")
                    and si is not None and len(si.on_wait) > limit):
                ename = str(inst.engine).split(".")[-1]
                waits = [copy_wait(w) for w in si.on_wait]
                upds = list(si.on_update)
                extra, keep = waits[:-limit], waits[-limit:]
                for w in extra:
                    insts.insert(i, make_nop(ename, [w]))
                    i += 1
                inst.sync_info = bass_rust.SyncInfo(
                    on_wait=keep, on_update=upds)
            i += 1


def _make_wmat(w):
    """128x128 stationary butterfly matrix, normalized to +-1 entries:
    W[(dt,dh,dw,g), (k,g)] = w[k,dt,dh,dw]/wamax.  Exact in fp16 for the
    Haar +-SCALE filters; works for any 8-filter 2x2x2 kernel."""
    w8 = np.asarray(w, dtype=np.float32).reshape(8, 2, 2, 2)
    wamax = float(np.abs(w8).max())
    if wamax == 0.0:
        wamax = 1.0
    wm = np.zeros((128, 128), dtype=np.float32)
    g = np.arange(16)
    for k in range(8):
        for dt in range(2):
            for dh in range(2):
                for dw in range(2):
                    wm[dt * 64 + dh * 32 + dw * 16 + g, k * 16 + g] = \
                        w8[k, dt, dh, dw] / wamax
    return wm.astype(np.float16), wamax


def _pack_input(xq):
    """xq (2,3,33,512,512) int8 -> list of 8 (128, NCOL) int8 per-core arrays."""
    pairs = np.empty((T_OUT, 2), dtype=np.int64)
    for t in range(T_OUT):
        pairs[t, 0] = max(2 * t - 1, 0)
        pairs[t, 1] = 2 * t
    full = xq[:, :, pairs]                        # (b, c, t, dt, 512, 512)
    # h = s*128 + g*8 + r*2 + dh ; w = wo*2 + dw
    arr = full.reshape(B, C, T_OUT, 2, 4, 16, 4, 2, 256, 2)
    #                  b  c  t     dt s  g   r  dh wo  dw
    arr = arr.transpose(0, 1, 2, 4, 3, 7, 9, 5, 6, 8)
    # (b, c, t, s, dt, dh, dw, g, r, wo)
    arr = np.ascontiguousarray(arr).reshape(8 * QU_PER_CORE, 128, 1024)
    return [
        np.ascontiguousarray(
            arr[QU_PER_CORE * m:QU_PER_CORE * (m + 1)].transpose(1, 0, 2)
        ).reshape(128, NCOL)
        for m in range(N_CORES)
    ]


def _unpack_output(youts, d_out):
    """list of 8 (128, NCOL) int8 -> (2, 24, 17, 256, 256) f32."""
    Y = np.stack(youts)                           # (8, 128, NCOL)
    arr = Y.reshape(N_CORES, 128, QU_PER_CORE, 1024).transpose(0, 2, 1, 3)
    arr = arr.reshape(B, C, T_OUT, 4, 8, 16, 4, 256)
    #                 b  c  t     s  k  g   r  wo
    arr = arr.transpose(0, 4, 1, 2, 3, 5, 6, 7)
    # (b, k, c, t, s, g, r, wo): channel = k*3+c, ho = s*64 + g*4 + r
    out = np.ascontiguousarray(arr).reshape(
        B, 24, T_OUT, 256, 256).astype(np.float32)
    out *= np.float32(d_out)
    return out


LAST_RESULT = None


def kernel(x, w):
    import os
    from concourse.bass_utils import run_bass_kernel_spmd

    x = np.asarray(x, dtype=np.float32)
    ax = float(np.abs(x).max())
    if ax == 0.0:
        ax = 1.0
    d_in = ax / 127.0
    xq = np.rint(x * np.float32(1.0 / d_in)).astype(np.int8)

    wm, wamax = _make_wmat(w)

    # Exact bound on |sum of +-xq| over any output window: max sum-abs pool
    # of the quantized input (with the causal first-frame replication).
    xa = np.abs(xq.astype(np.int16))
    pad = np.concatenate([xa[:, :, :1], xa], axis=2)      # (2,3,34,512,512)
    Bint = int(pad.reshape(B, C, T_OUT, 2, 256, 2, 256, 2)
               .sum(axis=(3, 5, 7), dtype=np.int32).max())
    if Bint == 0:
        Bint = 1
    alpha = np.float32(127.0 / Bint)
    d_out = wamax * d_in * Bint / 127.0

    in_maps = [{"xin": xc, "wmat": wm} for xc in _pack_input(xq)]

    nc = _build_nc(float(alpha))
    kw = {}
    if os.environ.get("KERNEL_PROFILE") == "1":
        kw = dict(trace=True, tmpdir=os.environ.get("KERNEL_PROFILE_DIR"))
    res = run_bass_kernel_spmd(nc, in_maps, core_ids=list(range(N_CORES)), **kw)
    global LAST_RESULT
    LAST_RESULT = res

    return _unpack_output(
        [np.asarray(res.results[m]["yout"]) for m in range(N_CORES)], d_out)


if __name__ == "__main__":
    rng = np.random.default_rng(0)
    x = rng.standard_normal((B, C, T_IN, H, W), dtype=np.float32)
    SCALE = 0.3536
    flags = np.array([[0, 0, 0], [0, 0, 1], [0, 1, 0], [0, 1, 1],
                      [1, 0, 0], [1, 0, 1], [1, 1, 0], [1, 1, 1]])
    t, h, ww = np.meshgrid(np.arange(2), np.arange(2), np.arange(2), indexing="ij")
    sign = (-1.0) ** (flags[:, 0, None, None, None] * t
                      + flags[:, 1, None, None, None] * h
                      + flags[:, 2, None, None, None] * ww)
    wf = (SCALE * sign).reshape(8, 1, 2, 2, 2).astype(np.float32)
    y = kernel(x, wf)
    print(y.shape, y.dtype)
